# revision 8
# baseline (speedup 1.0000x reference)
"""Trainium2 Bass kernel for nn_EncoderLayer (dense transformer encoder layer).

Sharding: data-parallel over batch. B=8 batch elements -> one per NeuronCore,
no collectives. Each core computes the full encoder layer for its batch row.

Dtype strategy (PE cost model: bf16/f32r = 1 cycle/row, fp8 DoubleRow = 0.5
cycles/row with two contraction tiles fused per instruction => 4x per MAC):
  - Q/K projections + attention scores in bf16 (exp() amplifies score error,
    fp8 there fails the error budget).
  - P = exp(S) stored bf16 (range reaches e^15, needs bf16 exponent), then
    self-normalized: P'' = P * 64/den written as fp8e4. P'' <= 64 < fp8max
    unconditionally, so no per-row max pass is needed.
  - den via ones-matmul over bf16 P; den2 = sum of the actual quantized P''
    via fp8-DoubleRow ones-matmul exactly renormalizes the attention weights.
  - attn@V, V projection, and the per-head gate Linear run fp8-DoubleRow.
  - Cross-head softmax accumulators in fp16 (fits range, 2-byte dtype keeps
    the fast DVE modes); fc in fp16.
Engine split keeps DVE/Act/GpSimd each under the PE critical path.
"""

import sys

sys.path.insert(0, "/opt/trn_rl_repo")

import contextlib

import numpy as np
import ml_dtypes

import concourse.bass as bass
import concourse.mybir as mybir
import concourse.tile as tile
from concourse import bass_utils

F32 = mybir.dt.float32
F32R = mybir.dt.float32r
BF16 = mybir.dt.bfloat16
F16 = mybir.dt.float16
F8 = mybir.dt.float8e4
EXP = mybir.ActivationFunctionType.Exp
DR = mybir.MatmulPerfMode.DoubleRow

B, L, DM, H, DK, DV = 8, 1024, 512, 8, 64, 512
P = 128
LT = L // P          # 8 key/query tiles of 128
KT4 = DM // P        # 4 contraction tiles over d_model
QC = L // 512        # 2 q-chunks of 512
NCORES = 8

_CACHE = {}


def build_nc(use_bias):
    nc = bass.Bass("TRN2", target_bir_lowering=False, debug=False)

    # Per-core inputs
    xtb_d = nc.dram_tensor("xtb", [DM, L], BF16, kind="ExternalInput")
    xt8_d = nc.dram_tensor("xt8", [DM, L], F8, kind="ExternalInput")
    x_d = nc.dram_tensor("x", [L, DM], F32, kind="ExternalInput")
    mb_d = nc.dram_tensor("mb", [P, LT], F32, kind="ExternalInput")
    np_d = nc.dram_tensor("npv", [P, LT], F32, kind="ExternalInput")
    # Shared weights (replicated on every core)
    wq_d = nc.dram_tensor("wqT", [DM, H * DK], BF16, kind="ExternalInput")
    wk_d = nc.dram_tensor("wkT", [DM, H * DK], BF16, kind="ExternalInput")
    wv_d = nc.dram_tensor("wvT", [DM, H * DV], F8, kind="ExternalInput")
    wg_d = nc.dram_tensor("wgT", [H, DM, DV], F8, kind="ExternalInput")
    wf_d = nc.dram_tensor("wfcT", [DV, DM], F16, kind="ExternalInput")
    if use_bias:
        bq_d = nc.dram_tensor("bq", [H, DK], F32, kind="ExternalInput")
        bk_d = nc.dram_tensor("bk", [H, DK], F32, kind="ExternalInput")
        bv_d = nc.dram_tensor("bv", [1, H * DV], BF16, kind="ExternalInput")
        bg_d = nc.dram_tensor("bg", [H * KT4, P], F32, kind="ExternalInput")
        bf_d = nc.dram_tensor("bfc", [1, DM], F16, kind="ExternalInput")
    y_d = nc.dram_tensor("y", [L, DM], F32, kind="ExternalOutput")

    with tile.TileContext(nc) as tc:
        with contextlib.ExitStack() as ctx:
            cpool = ctx.enter_context(tc.tile_pool(name="const", bufs=1))
            wqk_pool = ctx.enter_context(tc.tile_pool(name="wqk", bufs=2))
            wbig_pool = ctx.enter_context(tc.tile_pool(name="wbig", bufs=2))
            qk_pool = ctx.enter_context(tc.tile_pool(name="qk", bufs=2))
            v_pool = ctx.enter_context(tc.tile_pool(name="v", bufs=2))
            pt_pool = ctx.enter_context(tc.tile_pool(name="pt", bufs=2))
            p8_pool = ctx.enter_context(tc.tile_pool(name="p8", bufs=2))
            rden_pool = ctx.enter_context(tc.tile_pool(name="rden", bufs=2))
            ot_pool = ctx.enter_context(tc.tile_pool(name="ot", bufs=2))
            sm_pool = ctx.enter_context(tc.tile_pool(name="sm", bufs=4))
            io_pool = ctx.enter_context(tc.tile_pool(name="io", bufs=4))
            # PSUM: big pool = [128,1024] f32 tiles (2 banks each, bufs=2 ->
            # 4 banks) for scores+gate; small pool = [128,512] (bufs=4 ->
            # 4 banks) for everything else. Total exactly 8 banks.
            psb_pool = ctx.enter_context(
                tc.tile_pool(name="psb", bufs=2, space="PSUM")
            )
            pss_pool = ctx.enter_context(
                tc.tile_pool(name="pss", bufs=4, space="PSUM")
            )

            ones_bf = cpool.tile([P, P], BF16, tag="ones_bf")
            ones8 = cpool.tile([P, 2, P], F8, tag="ones8")
            ones_f = cpool.tile([P, P], F32, tag="ones_f")
            nc.gpsimd.memset(ones_f[:], 1.0)
            nc.vector.tensor_copy(ones_bf[:], ones_f[:])
            nc.vector.tensor_copy(ones8[:, 0, :], ones_f[:])
            nc.vector.tensor_copy(ones8[:, 1, :], ones_f[:])
            # den matmul uses 1/64-valued "ones" so the reciprocal directly
            # yields 64/den (the P'' scale) with no extra scalar op.
            ones_i64 = cpool.tile([P, P], BF16, tag="ones_i64")
            nc.vector.tensor_scalar_mul(ones_i64[:], ones_f[:], 0.015625)

            mb = cpool.tile([P, LT], F32, tag="mb")
            nc.sync.dma_start(mb[:], mb_d.ap())
            npv = cpool.tile([P, LT], F32, tag="npv")
            nc.sync.dma_start(npv[:], np_d.ap())

            xtb = cpool.tile([P, KT4, L], BF16, tag="xtb")
            xt8 = cpool.tile([P, KT4, L], F8, tag="xt8")
            for kt in range(KT4):
                for half in range(2):
                    sl = slice(half * 512, (half + 1) * 512)
                    nc.sync.dma_start(
                        xtb[:, kt, sl],
                        xtb_d.ap()[kt * P:(kt + 1) * P, sl],
                    )
                    nc.sync.dma_start(
                        xt8[:, kt, sl],
                        xt8_d.ap()[kt * P:(kt + 1) * P, sl],
                    )

            wfc = cpool.tile([P, KT4, DM], F16, tag="wfc")

            acc_n = cpool.tile([P, KT4, L], F16, tag="accn")
            acc_d = cpool.tile([P, KT4, L], F16, tag="accd")

            if use_bias:
                bq = cpool.tile([DK, H], F32, tag="bq")
                bk = cpool.tile([DK, H], F32, tag="bk")
                for h in range(H):
                    nc.sync.dma_start(
                        bq[:, h:h + 1], bq_d.ap()[h:h + 1, :].transpose([1, 0])
                    )
                    nc.sync.dma_start(
                        bk[:, h:h + 1], bk_d.ap()[h:h + 1, :].transpose([1, 0])
                    )
                bv = cpool.tile([1, H * DV], BF16, tag="bv")
                nc.sync.dma_start(bv[:], bv_d.ap())
                bg = cpool.tile([P, H * KT4], F32, tag="bg")
                for c in range(H * KT4):
                    nc.sync.dma_start(
                        bg[:, c:c + 1], bg_d.ap()[c:c + 1, :].transpose([1, 0])
                    )
                bf = cpool.tile([1, DM], F16, tag="bfc")
                nc.sync.dma_start(bf[:], bf_d.ap())

            for h in range(H):
                # ---- per-head weight slices ----
                wq = wqk_pool.tile([P, KT4, DK], BF16, tag="wq")
                wk = wqk_pool.tile([P, KT4, DK], BF16, tag="wk")
                for kt in range(KT4):
                    nc.sync.dma_start(
                        wq[:, kt, :],
                        wq_d.ap()[kt * P:(kt + 1) * P, h * DK:(h + 1) * DK],
                    )
                    nc.sync.dma_start(
                        wk[:, kt, :],
                        wk_d.ap()[kt * P:(kt + 1) * P, h * DK:(h + 1) * DK],
                    )
                wv8 = wbig_pool.tile([P, KT4, DV], F8, tag="wv8")
                wg8 = wbig_pool.tile([P, KT4, DM], F8, tag="wg8")

                # ---- Q.T, K.T : [DK, L] bf16, d_k on partitions ----
                qt = qk_pool.tile([DK, L], BF16, tag="qt")
                kt_sb = qk_pool.tile([DK, L], BF16, tag="kt")
                for qc in range(QC):
                    sl = slice(qc * 512, (qc + 1) * 512)
                    psA = pss_pool.tile([P, 512], F32, tag="pss")
                    for kt in range(KT4):
                        nc.tensor.matmul(
                            psA[0:DK, :],
                            wq[:, kt, :],
                            xtb[:, kt, sl],
                            start=(kt == 0),
                            stop=(kt == KT4 - 1),
                        )
                    if use_bias:
                        nc.vector.tensor_scalar(
                            qt[:, sl], psA[0:DK, :], bq[:, h:h + 1], 0.125,
                            mybir.AluOpType.add, mybir.AluOpType.mult,
                        )
                    else:
                        nc.vector.tensor_scalar_mul(qt[:, sl], psA[0:DK, :], 0.125)
                    psB = pss_pool.tile([P, 512], F32, tag="pss")
                    for kt in range(KT4):
                        nc.tensor.matmul(
                            psB[0:DK, :],
                            wk[:, kt, :],
                            xtb[:, kt, sl],
                            start=(kt == 0),
                            stop=(kt == KT4 - 1),
                        )
                    if use_bias:
                        nc.vector.tensor_scalar_add(kt_sb[:, sl], psB[0:DK, :], bk[:, h:h + 1])
                    else:
                        nc.vector.tensor_copy(kt_sb[:, sl], psB[0:DK, :])

                # ---- V : [L, DV] fp8, keys on partitions (fp8 DoubleRow) ----
                for kt in range(KT4):
                    nc.sync.dma_start(
                        wv8[:, kt, :],
                        wv_d.ap()[kt * P:(kt + 1) * P, h * DV:(h + 1) * DV],
                    )
                v8 = v_pool.tile([P, LT, DV], F8, tag="v8")
                for lt in range(LT):
                    psV = pss_pool.tile([P, 512], F32, tag="pss")
                    for ch in range(2):
                        csl = slice(ch * 256, (ch + 1) * 256)
                        for j in range(2):
                            nc.tensor.matmul(
                                psV[:, csl],
                                xt8[:, 2 * j:2 * j + 2, lt * P:(lt + 1) * P],
                                wv8[:, 2 * j:2 * j + 2, csl],
                                start=(j == 0),
                                stop=(j == 1 and not use_bias),
                                perf_mode=DR,
                            )
                        if use_bias:
                            nc.tensor.matmul(
                                psV[:, csl],
                                ones_bf[0:1, :],
                                bv[0:1, h * DV + ch * 256: h * DV + (ch + 1) * 256],
                                start=False,
                                stop=True,
                            )
                    nc.vector.tensor_copy(v8[:, lt, :], psV[:])

                # ---- P = exp(S + mask) bf16 : [L(keys), L(q)] ----
                pt = pt_pool.tile([P, LT, L], BF16, tag="pt")
                for ktile in range(LT):
                    psS = psb_pool.tile([P, L], F32, tag="psb")
                    ksl = slice(ktile * P, (ktile + 1) * P)
                    for qc in range(QC):
                        nc.tensor.matmul(
                            psS[:, qc * 512:(qc + 1) * 512],
                            kt_sb[:, ksl],
                            qt[:, qc * 512:(qc + 1) * 512],
                            start=True,
                            stop=True,
                        )
                    nc.scalar.activation(
                        pt[:, ktile, :], psS[:], EXP, bias=mb[:, ktile:ktile + 1]
                    )

                # ---- softmax denominator from bf16 P ((1/64)-matmul) ----
                rden = rden_pool.tile([P, L], F32, tag="rden")
                for qc in range(QC):
                    sl = slice(qc * 512, (qc + 1) * 512)
                    psD = pss_pool.tile([P, 512], F32, tag="pss")
                    for ktile in range(LT):
                        nc.tensor.matmul(
                            psD[:],
                            ones_i64[:],
                            pt[:, ktile, sl],
                            start=(ktile == 0),
                            stop=(ktile == LT - 1),
                        )
                    nc.vector.reciprocal(rden[:, sl], psD[:])

                # ---- P'' = P * 64/den -> fp8e4 (mostly GpSimd; SBUF-only) ----
                p8 = p8_pool.tile([P, LT, L], F8, tag="p8")
                for ktile in range(LT):
                    eng = nc.vector if ktile >= 6 else nc.gpsimd
                    eng.tensor_tensor(
                        p8[:, ktile, :], pt[:, ktile, :], rden[:],
                        mybir.AluOpType.mult,
                    )

                # ---- den2 = sum of quantized P'' (fp8 DR ones matmul) ----
                rden2 = rden_pool.tile([P, L], F32, tag="rden2")
                for qc in range(QC):
                    sl = slice(qc * 512, (qc + 1) * 512)
                    psD2 = pss_pool.tile([P, 512], F32, tag="pss")
                    for ch in range(2):
                        csl = slice(ch * 256, (ch + 1) * 256)
                        gsl = slice(qc * 512 + ch * 256, qc * 512 + (ch + 1) * 256)
                        for j in range(4):
                            nc.tensor.matmul(
                                psD2[:, csl],
                                ones8[:],
                                p8[:, 2 * j:2 * j + 2, gsl],
                                start=(j == 0),
                                stop=(j == 3),
                                perf_mode=DR,
                            )
                    nc.vector.reciprocal(rden2[:, sl], psD2[:])

                # ---- O.T = V.T @ P'' (fp8 DR), renormalized by den2 ----
                o16 = ot_pool.tile([P, KT4, L], F16, tag="o16")
                o8 = ot_pool.tile([P, KT4, L], F8, tag="o8")
                for dv in range(KT4):
                    dsl = slice(dv * P, (dv + 1) * P)
                    for qc in range(QC):
                        sl = slice(qc * 512, (qc + 1) * 512)
                        psO = pss_pool.tile([P, 512], F32, tag="pss")
                        for ch in range(2):
                            csl = slice(ch * 256, (ch + 1) * 256)
                            gsl = slice(qc * 512 + ch * 256, qc * 512 + (ch + 1) * 256)
                            for j in range(4):
                                nc.tensor.matmul(
                                    psO[:, csl],
                                    v8[:, 2 * j:2 * j + 2, dsl],
                                    p8[:, 2 * j:2 * j + 2, gsl],
                                    start=(j == 0),
                                    stop=(j == 3),
                                    perf_mode=DR,
                                )
                        nc.vector.tensor_tensor(
                            o16[:, dv, sl], psO[:], rden2[:, sl],
                            mybir.AluOpType.mult,
                        )
                        # fp8 copy for the gate matmul (SBUF->SBUF on GpSimd;
                        # GPSIMD cannot read PSUM)
                        nc.gpsimd.tensor_copy(o8[:, dv, sl], o16[:, dv, sl])

                # ---- gate logits (fp8 DR) + exp + cross-head accumulation ----
                for kt in range(KT4):
                    nc.sync.dma_start(
                        wg8[:, kt, :],
                        wg_d.ap()[h, kt * P:(kt + 1) * P, :],
                    )
                for et in range(KT4):
                    esl = slice(et * P, (et + 1) * P)
                    psG = psb_pool.tile([P, L], F32, tag="psb")
                    for qc in range(QC):
                        for ch in range(2):
                            gsl = slice(qc * 512 + ch * 256, qc * 512 + (ch + 1) * 256)
                            for j in range(2):
                                nc.tensor.matmul(
                                    psG[:, gsl],
                                    wg8[:, 2 * j:2 * j + 2, esl],
                                    o8[:, 2 * j:2 * j + 2, gsl],
                                    start=(j == 0),
                                    stop=(j == 1),
                                    perf_mode=DR,
                                )
                    gx = sm_pool.tile([P, L], F16, tag="gx")
                    if use_bias:
                        nc.scalar.activation(
                            gx[:], psG[:], EXP,
                            bias=bg[:, h * KT4 + et: h * KT4 + et + 1],
                        )
                    else:
                        nc.scalar.activation(gx[:], psG[:], EXP)
                    with nc.allow_low_precision(
                        reason="fp16 cross-head accumulators: 8 summands of "
                        "comparable magnitude, rel err ~1e-3 << budget"
                    ):
                        if h == 0:
                            nc.vector.tensor_tensor(
                                acc_n[:, et, :], gx[:], o16[:, et, :],
                                mybir.AluOpType.mult,
                            )
                            nc.vector.tensor_copy(acc_d[:, et, :], gx[:])
                        else:
                            tm = sm_pool.tile([P, L], F16, tag="tm")
                            nc.vector.tensor_tensor(
                                tm[:], gx[:], o16[:, et, :], mybir.AluOpType.mult,
                            )
                            nc.vector.tensor_add(acc_n[:, et, :], acc_n[:, et, :], tm[:])
                            nc.vector.tensor_add(acc_d[:, et, :], acc_d[:, et, :], gx[:])
                        if h == H - 1:
                            rc = sm_pool.tile([P, L], F16, tag="rc")
                            nc.vector.reciprocal(rc[:], acc_d[:, et, :])
                            nc.vector.tensor_tensor(
                                acc_n[:, et, :], acc_n[:, et, :], rc[:],
                                mybir.AluOpType.mult,
                            )

            # ---- fc + residual + nonpad zeroing : y[q, m] natural ----
            for et in range(KT4):
                nc.sync.dma_start(
                    wfc[:, et, :], wf_d.ap()[et * P:(et + 1) * P, :]
                )
            for qt8 in range(LT):
                ps = pss_pool.tile([P, 512], F32, tag="pss")
                for et in range(KT4):
                    nc.tensor.matmul(
                        ps[:],
                        acc_n[:, et, qt8 * P:(qt8 + 1) * P],
                        wfc[:, et, :],
                        start=(et == 0),
                        stop=(et == KT4 - 1 and not use_bias),
                    )
                if use_bias:
                    nc.tensor.matmul(
                        ps[:], ones_bf[0:1, :], bf[0:1, :], start=False, stop=True,
                    )
                xres = io_pool.tile([P, DM], F32, tag="xres")
                nc.sync.dma_start(xres[:], x_d.ap()[qt8 * P:(qt8 + 1) * P, :])
                ysb = io_pool.tile([P, DM], F32, tag="ysb")
                nc.vector.scalar_tensor_tensor(
                    ysb[:], ps[:], npv[:, qt8:qt8 + 1], xres[:],
                    mybir.AluOpType.mult, mybir.AluOpType.add,
                )
                nc.sync.dma_start(y_d.ap()[qt8 * P:(qt8 + 1) * P, :], ysb[:])

    split_multi_waits(nc)
    return nc


def split_multi_waits(nc):
    """This env's walrus only allows one sync-wait per instruction; hoist
    extra waits onto NoOps inserted just before, on the same engine."""
    n_fix = 0
    for f in nc.m.functions:
        for bb in f.blocks:
            insts = bb.instructions
            out = []
            changed = False
            for ins in insts:
                si = ins.sync_info
                if si is not None and len(si.on_wait) > 1:
                    waits = list(si.on_wait)
                    for k, w in enumerate(waits[:-1]):
                        nop = mybir.InstNoOp(
                            name=f"{ins.name}-waitsplit{k}",
                            engine=ins.engine,
                            ins=[],
                            outs=[],
                            sync_info=mybir.SyncInfo(on_wait=[w], on_update=[]),
                        )
                        out.append(nop)
                    ins.sync_info = mybir.SyncInfo(
                        on_wait=[waits[-1]], on_update=list(si.on_update)
                    )
                    changed = True
                    n_fix += 1
                out.append(ins)
            if changed:
                bb.instructions = out
    return n_fix


def _prep_inputs(enc_input, non_pad_mask, slf_attn_mask,
                 w_q, b_q, w_k, b_k, w_v, b_v, w_gate, b_gate, w_fc, b_fc,
                 use_bias):
    f32 = np.float32
    bf16 = ml_dtypes.bfloat16
    f16 = np.float16
    f8 = ml_dtypes.float8_e4m3
    shared = {
        "wqT": np.ascontiguousarray(np.asarray(w_q).T).astype(bf16),
        "wkT": np.ascontiguousarray(np.asarray(w_k).T).astype(bf16),
        "wvT": np.ascontiguousarray(np.asarray(w_v).T).astype(f8),
        "wgT": np.ascontiguousarray(
            np.asarray(w_gate).transpose(0, 2, 1)
        ).astype(f8),
        "wfcT": np.ascontiguousarray(np.asarray(w_fc).T).astype(f16),
    }
    if use_bias:
        shared["bq"] = np.ascontiguousarray(b_q.reshape(H, DK), dtype=f32)
        shared["bk"] = np.ascontiguousarray(b_k.reshape(H, DK), dtype=f32)
        shared["bv"] = np.ascontiguousarray(b_v.reshape(1, H * DV)).astype(bf16)
        shared["bg"] = np.ascontiguousarray(
            b_gate.reshape(H * KT4, P), dtype=f32
        )
        shared["bfc"] = np.ascontiguousarray(b_fc.reshape(1, DM)).astype(f16)

    in_maps = []
    for b in range(B):
        key_pad = np.asarray(slf_attn_mask[b, 0, :])
        mb = np.where(key_pad, f32(-30000.0), f32(0.0)).astype(f32)
        q_pad = np.asarray(non_pad_mask[b, :, 0])
        npv = np.where(q_pad, f32(0.0), f32(1.0)).astype(f32)
        xt = np.ascontiguousarray(np.asarray(enc_input[b]).T, dtype=f32)
        m = {
            "xtb": xt.astype(bf16),
            "xt8": xt.astype(f8),
            "x": np.ascontiguousarray(enc_input[b] * npv[:, None], dtype=f32),
            "mb": np.ascontiguousarray(mb.reshape(LT, P).T),
            "npv": np.ascontiguousarray(npv.reshape(LT, P).T),
        }
        m.update(shared)
        in_maps.append(m)
    return in_maps


def kernel(enc_input, non_pad_mask, slf_attn_mask,
           w_q, b_q, w_k, b_k, w_v, b_v, w_gate, b_gate, w_fc, b_fc,
           **_unused):
    enc_input = np.asarray(enc_input)
    assert enc_input.shape == (B, L, DM)
    use_bias = any(
        np.any(np.asarray(a)) for a in (b_q, b_k, b_v, b_gate, b_fc)
    )

    if use_bias not in _CACHE:
        _CACHE[use_bias] = build_nc(use_bias)
    nc = _CACHE[use_bias]

    in_maps = _prep_inputs(
        enc_input, non_pad_mask, slf_attn_mask,
        w_q, b_q, w_k, b_k, w_v, b_v, w_gate, b_gate, w_fc, b_fc, use_bias,
    )
    res = bass_utils.run_bass_kernel_spmd(nc, in_maps, core_ids=list(range(NCORES)))
    out = np.stack([res.results[b]["y"] for b in range(B)], axis=0)
    return out.astype(np.float32)


# revision 9
# speedup vs baseline: 1.1803x; 1.1803x over previous
"""Trainium2 Bass kernel for nn_EncoderLayer (dense transformer encoder layer).

Sharding: data-parallel over batch. B=8 batch elements -> one per NeuronCore,
no collectives. Each core computes the full encoder layer for its batch row.

Dtype strategy (PE cost: bf16/f32r = 1 cycle/row; fp8 DoubleRow = 0.5
cycles/row with two 128-deep contraction tiles fused per instruction):
  - Q/K projections + attention scores in bf16 (exp() amplifies score error;
    fp8 there blows the error budget).
  - P = exp(S) stored bf16 (needs bf16 exponent range), then self-normalized:
    P'' = P * 64/den written as fp8e4. P'' <= 64 < fp8max unconditionally, so
    no per-row max pass is needed. den comes from a (1/64)-valued ones-matmul
    so one reciprocal directly yields the P'' scale.
  - den2 = sum of the actual quantized P'' (fp8-DR ones-matmul) exactly
    renormalizes the attention weights after quantization.
  - attn@V, V projection, and the per-head gate Linear run fp8-DoubleRow.
  - Cross-head softmax accumulators in fp16 (2-byte dtype keeps the fast DVE
    modes); fc in fp16.

Schedule: software-pipelined across heads. While GpSimd/DVE quantize P'' of
head h, the PE runs head h+1's QKV projections; head h+1's score matmuls fill
the PE gap between attn@V and the gate matmul of head h. All weight loads are
single merged DMAs (HWDGE issue cost is flat per descriptor set).
"""

import sys

sys.path.insert(0, "/opt/trn_rl_repo")

import contextlib

import numpy as np
import ml_dtypes

import concourse.bass as bass
import concourse.mybir as mybir
import concourse.tile as tile
from concourse import bass_utils

F32 = mybir.dt.float32
BF16 = mybir.dt.bfloat16
F16 = mybir.dt.float16
F8 = mybir.dt.float8e4
EXP = mybir.ActivationFunctionType.Exp
DR = mybir.MatmulPerfMode.DoubleRow
MUL = mybir.AluOpType.mult

B, L, DM, H, DK, DV = 8, 1024, 512, 8, 64, 512
P = 128
LT = L // P          # 8 key/query tiles of 128
KT4 = DM // P        # 4 contraction tiles over d_model
QC = L // 512        # 2 q-chunks of 512
NCORES = 8

_CACHE = {}


def build_nc(use_bias):
    nc = bass.Bass("TRN2", target_bir_lowering=False, debug=False)

    # Per-core inputs (leading dim = d_model tile so one DMA loads a whole
    # [128, KT4, *] SBUF tile via a transposed access pattern)
    xtb_d = nc.dram_tensor("xtb", [KT4, P, L], BF16, kind="ExternalInput")
    xt8_d = nc.dram_tensor("xt8", [KT4, P, L], F8, kind="ExternalInput")
    x_d = nc.dram_tensor("x", [L, DM], F32, kind="ExternalInput")
    mb_d = nc.dram_tensor("mb", [P, LT], F32, kind="ExternalInput")
    np_d = nc.dram_tensor("npv", [P, LT], F32, kind="ExternalInput")
    wq_d = nc.dram_tensor("wqT", [KT4, P, H * DK], BF16, kind="ExternalInput")
    wk_d = nc.dram_tensor("wkT", [KT4, P, H * DK], BF16, kind="ExternalInput")
    wv_d = nc.dram_tensor("wvT", [KT4, P, H * DV], F8, kind="ExternalInput")
    wg_d = nc.dram_tensor("wgT", [H * KT4, P, DV], F8, kind="ExternalInput")
    wf_d = nc.dram_tensor("wfcT", [KT4, P, DM], F16, kind="ExternalInput")
    if use_bias:
        bq_d = nc.dram_tensor("bq", [H, DK], F32, kind="ExternalInput")
        bk_d = nc.dram_tensor("bk", [H, DK], F32, kind="ExternalInput")
        bv_d = nc.dram_tensor("bv", [1, H * DV], BF16, kind="ExternalInput")
        bg_d = nc.dram_tensor("bg", [H * KT4, P], F32, kind="ExternalInput")
        bf_d = nc.dram_tensor("bfc", [1, DM], F16, kind="ExternalInput")
    y_d = nc.dram_tensor("y", [L, DM], F32, kind="ExternalOutput")

    with tile.TileContext(nc) as tc:
        with contextlib.ExitStack() as ctx:
            cpool = ctx.enter_context(tc.tile_pool(name="const", bufs=1))
            wqk_pool = ctx.enter_context(tc.tile_pool(name="wqk", bufs=2))
            wbig_pool = ctx.enter_context(tc.tile_pool(name="wbig", bufs=2))
            qk_pool = ctx.enter_context(tc.tile_pool(name="qk", bufs=2))
            v_pool = ctx.enter_context(tc.tile_pool(name="v", bufs=2))
            pt_pool = ctx.enter_context(tc.tile_pool(name="pt", bufs=2))
            p8_pool = ctx.enter_context(tc.tile_pool(name="p8", bufs=2))
            rden_pool = ctx.enter_context(tc.tile_pool(name="rden", bufs=2))
            ot_pool = ctx.enter_context(tc.tile_pool(name="ot", bufs=2))
            sm_pool = ctx.enter_context(tc.tile_pool(name="sm", bufs=4))
            io_pool = ctx.enter_context(tc.tile_pool(name="io", bufs=4))
            # PSUM: psb = [128,1024] tiles (2 banks, bufs=2 -> 4 banks) for
            # scores+gate; pss = [128,512] (bufs=4 -> 4 banks) for the rest.
            psb_pool = ctx.enter_context(
                tc.tile_pool(name="psb", bufs=2, space="PSUM")
            )
            pss_pool = ctx.enter_context(
                tc.tile_pool(name="pss", bufs=4, space="PSUM")
            )

            ones8 = cpool.tile([P, 2, P], F8, tag="ones8")
            ones_f = cpool.tile([P, P], F32, tag="ones_f")
            nc.gpsimd.memset(ones_f[:], 1.0)
            nc.vector.tensor_copy(ones8[:, 0, :], ones_f[:])
            nc.vector.tensor_copy(ones8[:, 1, :], ones_f[:])
            # den matmul uses 1/64-valued "ones" so the reciprocal directly
            # yields 64/den (the P'' scale) with no extra scalar op.
            ones_i64 = cpool.tile([P, P], BF16, tag="ones_i64")
            nc.vector.tensor_scalar_mul(ones_i64[:], ones_f[:], 0.015625)
            if use_bias:
                ones_bf = cpool.tile([1, P], BF16, tag="ones_bf")
                nc.vector.tensor_copy(ones_bf[:], ones_f[0:1, :])

            mb = cpool.tile([P, LT], F32, tag="mb")
            nc.sync.dma_start(mb[:], mb_d.ap())
            npv = cpool.tile([P, LT], F32, tag="npv")
            nc.sync.dma_start(npv[:], np_d.ap())

            xtb = cpool.tile([P, KT4, L], BF16, tag="xtb")
            nc.sync.dma_start(xtb[:], xtb_d.ap().transpose([1, 0, 2]))
            xt8 = cpool.tile([P, KT4, L], F8, tag="xt8")
            nc.sync.dma_start(xt8[:], xt8_d.ap().transpose([1, 0, 2]))

            wfc = cpool.tile([P, KT4, DM], F16, tag="wfc")
            nc.sync.dma_start(wfc[:], wf_d.ap().transpose([1, 0, 2]))

            acc_n = cpool.tile([P, KT4, L], F16, tag="accn")
            acc_d = cpool.tile([P, KT4, L], F16, tag="accd")

            if use_bias:
                bq = cpool.tile([DK, H], F32, tag="bq")
                nc.sync.dma_start(bq[:], bq_d.ap().transpose([1, 0]))
                bk = cpool.tile([DK, H], F32, tag="bk")
                nc.sync.dma_start(bk[:], bk_d.ap().transpose([1, 0]))
                bv = cpool.tile([1, H * DV], BF16, tag="bv")
                nc.sync.dma_start(bv[:], bv_d.ap())
                bg = cpool.tile([P, H * KT4], F32, tag="bg")
                nc.sync.dma_start(bg[:], bg_d.ap().transpose([1, 0]))
                bf = cpool.tile([1, DM], F16, tag="bfc")
                nc.sync.dma_start(bf[:], bf_d.ap())

            def emit_qkv(h):
                """Weight DMAs + Q.T/K.T (bf16) + V (fp8 DR) for head h."""
                wq = wqk_pool.tile([P, KT4, DK], BF16, tag="wq")
                wk = wqk_pool.tile([P, KT4, DK], BF16, tag="wk")
                hk = slice(h * DK, (h + 1) * DK)
                nc.sync.dma_start(wq[:], wq_d.ap()[:, :, hk].transpose([1, 0, 2]))
                nc.sync.dma_start(wk[:], wk_d.ap()[:, :, hk].transpose([1, 0, 2]))
                wv8 = wbig_pool.tile([P, KT4, DV], F8, tag="wv8")
                hv = slice(h * DV, (h + 1) * DV)
                nc.sync.dma_start(wv8[:], wv_d.ap()[:, :, hv].transpose([1, 0, 2]))

                qt = qk_pool.tile([DK, L], BF16, tag="qt")
                kt_sb = qk_pool.tile([DK, L], BF16, tag="kt")
                for qc in range(QC):
                    sl = slice(qc * 512, (qc + 1) * 512)
                    psA = pss_pool.tile([P, 512], F32, tag="pss")
                    for kt in range(KT4):
                        nc.tensor.matmul(
                            psA[0:DK, :], wq[:, kt, :], xtb[:, kt, sl],
                            start=(kt == 0), stop=(kt == KT4 - 1),
                        )
                    if use_bias:
                        nc.vector.tensor_scalar(
                            qt[:, sl], psA[0:DK, :], bq[:, h:h + 1], 0.125,
                            mybir.AluOpType.add, MUL,
                        )
                    else:
                        nc.vector.tensor_scalar_mul(qt[:, sl], psA[0:DK, :], 0.125)
                    psB = pss_pool.tile([P, 512], F32, tag="pss")
                    for kt in range(KT4):
                        nc.tensor.matmul(
                            psB[0:DK, :], wk[:, kt, :], xtb[:, kt, sl],
                            start=(kt == 0), stop=(kt == KT4 - 1),
                        )
                    if use_bias:
                        nc.vector.tensor_scalar_add(
                            kt_sb[:, sl], psB[0:DK, :], bk[:, h:h + 1]
                        )
                    else:
                        nc.vector.tensor_copy(kt_sb[:, sl], psB[0:DK, :])

                v8 = v_pool.tile([P, LT, DV], F8, tag="v8")
                for lt in range(LT):
                    psV = pss_pool.tile([P, 512], F32, tag="pss")
                    for ch in range(2):
                        csl = slice(ch * 256, (ch + 1) * 256)
                        for j in range(2):
                            nc.tensor.matmul(
                                psV[:, csl],
                                xt8[:, 2 * j:2 * j + 2, lt * P:(lt + 1) * P],
                                wv8[:, 2 * j:2 * j + 2, csl],
                                start=(j == 0),
                                stop=(j == 1 and not use_bias),
                                perf_mode=DR,
                            )
                        if use_bias:
                            nc.tensor.matmul(
                                psV[:, csl],
                                ones_bf[0:1, :],
                                bv[0:1, h * DV + ch * 256: h * DV + (ch + 1) * 256],
                                start=False, stop=True,
                            )
                    # split PSUM->fp8 copies between DVE and Act
                    if lt % 2 == 0:
                        nc.vector.tensor_copy(v8[:, lt, :], psV[:])
                    else:
                        nc.scalar.copy(v8[:, lt, :], psV[:])
                return qt, kt_sb, v8

            def emit_scores(h, qt, kt_sb):
                """S = K.T^T@Q.T (bf16) + exp -> P bf16 for head h."""
                pt = pt_pool.tile([P, LT, L], BF16, tag="pt")
                for ktile in range(LT):
                    psS = psb_pool.tile([P, L], F32, tag="psb")
                    ksl = slice(ktile * P, (ktile + 1) * P)
                    for qc in range(QC):
                        nc.tensor.matmul(
                            psS[:, qc * 512:(qc + 1) * 512],
                            kt_sb[:, ksl],
                            qt[:, qc * 512:(qc + 1) * 512],
                            start=True, stop=True,
                        )
                    nc.scalar.activation(
                        pt[:, ktile, :], psS[:], EXP, bias=mb[:, ktile:ktile + 1]
                    )
                return pt

            def emit_den_p8(h, pt):
                """den from bf16 P; P'' = P*64/den -> fp8 (Pool + DVE)."""
                rden = rden_pool.tile([P, L], F32, tag="rden")
                for qc in range(QC):
                    sl = slice(qc * 512, (qc + 1) * 512)
                    psD = pss_pool.tile([P, 512], F32, tag="pss")
                    for ktile in range(LT):
                        nc.tensor.matmul(
                            psD[:], ones_i64[:], pt[:, ktile, sl],
                            start=(ktile == 0), stop=(ktile == LT - 1),
                        )
                    nc.vector.reciprocal(rden[:, sl], psD[:])
                p8 = p8_pool.tile([P, LT, L], F8, tag="p8")
                for ktile in range(LT):
                    eng = nc.vector if ktile >= 6 else nc.gpsimd
                    eng.tensor_tensor(
                        p8[:, ktile, :], pt[:, ktile, :], rden[:], MUL,
                    )
                return p8

            def emit_attnv(h, v8, p8):
                """den2 + O.T = V.T@P'' (fp8 DR), den2-normalized f16/fp8."""
                rden2 = rden_pool.tile([P, L], F32, tag="rden2")
                for qc in range(QC):
                    sl = slice(qc * 512, (qc + 1) * 512)
                    psD2 = pss_pool.tile([P, 512], F32, tag="pss")
                    for ch in range(2):
                        csl = slice(ch * 256, (ch + 1) * 256)
                        gsl = slice(qc * 512 + ch * 256, qc * 512 + (ch + 1) * 256)
                        for j in range(4):
                            nc.tensor.matmul(
                                psD2[:, csl], ones8[:], p8[:, 2 * j:2 * j + 2, gsl],
                                start=(j == 0), stop=(j == 3), perf_mode=DR,
                            )
                    nc.vector.reciprocal(rden2[:, sl], psD2[:])
                o16 = ot_pool.tile([P, KT4, L], F16, tag="o16")
                o8 = ot_pool.tile([P, KT4, L], F8, tag="o8")
                for qc in range(QC):
                    sl = slice(qc * 512, (qc + 1) * 512)
                    for dv in range(KT4):
                        dsl = slice(dv * P, (dv + 1) * P)
                        psO = pss_pool.tile([P, 512], F32, tag="pss")
                        for ch in range(2):
                            csl = slice(ch * 256, (ch + 1) * 256)
                            gsl = slice(qc * 512 + ch * 256, qc * 512 + (ch + 1) * 256)
                            for j in range(4):
                                nc.tensor.matmul(
                                    psO[:, csl],
                                    v8[:, 2 * j:2 * j + 2, dsl],
                                    p8[:, 2 * j:2 * j + 2, gsl],
                                    start=(j == 0), stop=(j == 3), perf_mode=DR,
                                )
                        nc.vector.tensor_tensor(
                            o16[:, dv, sl], psO[:], rden2[:, sl], MUL,
                        )
                        # fp8 copy for the gate matmul (GpSimd can't read PSUM)
                        nc.gpsimd.tensor_copy(o8[:, dv, sl], o16[:, dv, sl])
                return o16, o8

            def emit_gate(h, o16, o8):
                """Gate logits (fp8 DR) + exp + f16 cross-head accumulation."""
                wg8 = wbig_pool.tile([P, KT4, DM], F8, tag="wg8")
                nc.sync.dma_start(
                    wg8[:],
                    wg_d.ap()[h * KT4:(h + 1) * KT4, :, :].transpose([1, 0, 2]),
                )
                for et in range(KT4):
                    esl = slice(et * P, (et + 1) * P)
                    psG = psb_pool.tile([P, L], F32, tag="psb")
                    for qc in range(QC):
                        for ch in range(2):
                            gsl = slice(qc * 512 + ch * 256, qc * 512 + (ch + 1) * 256)
                            for j in range(2):
                                nc.tensor.matmul(
                                    psG[:, gsl],
                                    wg8[:, 2 * j:2 * j + 2, esl],
                                    o8[:, 2 * j:2 * j + 2, gsl],
                                    start=(j == 0), stop=(j == 1), perf_mode=DR,
                                )
                    gx = sm_pool.tile([P, L], F16, tag="gx")
                    if use_bias:
                        nc.scalar.activation(
                            gx[:], psG[:], EXP,
                            bias=bg[:, h * KT4 + et: h * KT4 + et + 1],
                        )
                    else:
                        nc.scalar.activation(gx[:], psG[:], EXP)
                    with nc.allow_low_precision(
                        reason="fp16 cross-head accumulators: 8 summands of "
                        "comparable magnitude, rel err ~1e-3 << budget"
                    ):
                        if h == 0:
                            nc.vector.tensor_tensor(
                                acc_n[:, et, :], gx[:], o16[:, et, :], MUL,
                            )
                            nc.vector.tensor_copy(acc_d[:, et, :], gx[:])
                        else:
                            tm = sm_pool.tile([P, L], F16, tag="tm")
                            nc.vector.tensor_tensor(
                                tm[:], gx[:], o16[:, et, :], MUL,
                            )
                            nc.vector.tensor_add(
                                acc_n[:, et, :], acc_n[:, et, :], tm[:]
                            )
                            nc.vector.tensor_add(
                                acc_d[:, et, :], acc_d[:, et, :], gx[:]
                            )
                        if h == H - 1:
                            rc = sm_pool.tile([P, L], F16, tag="rc")
                            nc.vector.reciprocal(rc[:], acc_d[:, et, :])
                            nc.vector.tensor_tensor(
                                acc_n[:, et, :], acc_n[:, et, :], rc[:], MUL,
                            )

            # ---- software-pipelined head loop ----
            qt, kt_sb, v8 = emit_qkv(0)
            pt = emit_scores(0, qt, kt_sb)
            state = (v8, pt)
            for h in range(H):
                v8, pt = state
                p8 = emit_den_p8(h, pt)
                # next head's QKV runs on the PE while Pool/DVE quantize P''
                if h + 1 < H:
                    qt_n, kt_n, v8_n = emit_qkv(h + 1)
                o16, o8 = emit_attnv(h, v8, p8)
                # next head's score matmuls fill the PE gap while DVE/GpSimd
                # write o16/o8 for the gate
                if h + 1 < H:
                    pt_n = emit_scores(h + 1, qt_n, kt_n)
                    state = (v8_n, pt_n)
                emit_gate(h, o16, o8)

            # ---- fc + residual + nonpad zeroing : y[q, m] natural ----
            for qt8 in range(LT):
                ps = pss_pool.tile([P, 512], F32, tag="pss")
                for et in range(KT4):
                    nc.tensor.matmul(
                        ps[:],
                        acc_n[:, et, qt8 * P:(qt8 + 1) * P],
                        wfc[:, et, :],
                        start=(et == 0),
                        stop=(et == KT4 - 1 and not use_bias),
                    )
                if use_bias:
                    nc.tensor.matmul(
                        ps[:], ones_bf[0:1, :], bf[0:1, :], start=False, stop=True,
                    )
                xres = io_pool.tile([P, DM], F32, tag="xres")
                nc.sync.dma_start(xres[:], x_d.ap()[qt8 * P:(qt8 + 1) * P, :])
                ysb = io_pool.tile([P, DM], F32, tag="ysb")
                nc.vector.scalar_tensor_tensor(
                    ysb[:], ps[:], npv[:, qt8:qt8 + 1], xres[:],
                    MUL, mybir.AluOpType.add,
                )
                nc.sync.dma_start(y_d.ap()[qt8 * P:(qt8 + 1) * P, :], ysb[:])

    split_multi_waits(nc)
    return nc


def split_multi_waits(nc):
    """This env's walrus only allows one sync-wait per instruction; hoist
    extra waits onto NoOps inserted just before, on the same engine."""
    n_fix = 0
    for f in nc.m.functions:
        for bb in f.blocks:
            insts = bb.instructions
            out = []
            changed = False
            for ins in insts:
                si = ins.sync_info
                if si is not None and len(si.on_wait) > 1:
                    waits = list(si.on_wait)
                    for k, w in enumerate(waits[:-1]):
                        nop = mybir.InstNoOp(
                            name=f"{ins.name}-waitsplit{k}",
                            engine=ins.engine,
                            ins=[],
                            outs=[],
                            sync_info=mybir.SyncInfo(on_wait=[w], on_update=[]),
                        )
                        out.append(nop)
                    ins.sync_info = mybir.SyncInfo(
                        on_wait=[waits[-1]], on_update=list(si.on_update)
                    )
                    changed = True
                    n_fix += 1
                out.append(ins)
            if changed:
                bb.instructions = out
    return n_fix


def _prep_inputs(enc_input, non_pad_mask, slf_attn_mask,
                 w_q, b_q, w_k, b_k, w_v, b_v, w_gate, b_gate, w_fc, b_fc,
                 use_bias):
    f32 = np.float32
    bf16 = ml_dtypes.bfloat16
    f16 = np.float16
    f8 = ml_dtypes.float8_e4m3
    shared = {
        "wqT": np.ascontiguousarray(np.asarray(w_q).T).astype(bf16).reshape(KT4, P, H * DK),
        "wkT": np.ascontiguousarray(np.asarray(w_k).T).astype(bf16).reshape(KT4, P, H * DK),
        "wvT": np.ascontiguousarray(np.asarray(w_v).T).astype(f8).reshape(KT4, P, H * DV),
        "wgT": np.ascontiguousarray(
            np.asarray(w_gate).transpose(0, 2, 1)
        ).astype(f8).reshape(H * KT4, P, DV),
        "wfcT": np.ascontiguousarray(np.asarray(w_fc).T).astype(f16).reshape(KT4, P, DM),
    }
    if use_bias:
        shared["bq"] = np.ascontiguousarray(b_q.reshape(H, DK), dtype=f32)
        shared["bk"] = np.ascontiguousarray(b_k.reshape(H, DK), dtype=f32)
        shared["bv"] = np.ascontiguousarray(b_v.reshape(1, H * DV)).astype(bf16)
        shared["bg"] = np.ascontiguousarray(
            b_gate.reshape(H * KT4, P), dtype=f32
        )
        shared["bfc"] = np.ascontiguousarray(b_fc.reshape(1, DM)).astype(f16)

    in_maps = []
    for b in range(B):
        key_pad = np.asarray(slf_attn_mask[b, 0, :])
        mb = np.where(key_pad, f32(-30000.0), f32(0.0)).astype(f32)
        q_pad = np.asarray(non_pad_mask[b, :, 0])
        npv = np.where(q_pad, f32(0.0), f32(1.0)).astype(f32)
        xt = np.ascontiguousarray(np.asarray(enc_input[b]).T, dtype=f32)
        m = {
            "xtb": xt.astype(bf16).reshape(KT4, P, L),
            "xt8": xt.astype(f8).reshape(KT4, P, L),
            "x": np.ascontiguousarray(enc_input[b] * npv[:, None], dtype=f32),
            "mb": np.ascontiguousarray(mb.reshape(LT, P).T),
            "npv": np.ascontiguousarray(npv.reshape(LT, P).T),
        }
        m.update(shared)
        in_maps.append(m)
    return in_maps


def kernel(enc_input, non_pad_mask, slf_attn_mask,
           w_q, b_q, w_k, b_k, w_v, b_v, w_gate, b_gate, w_fc, b_fc,
           **_unused):
    enc_input = np.asarray(enc_input)
    assert enc_input.shape == (B, L, DM)
    use_bias = any(
        np.any(np.asarray(a)) for a in (b_q, b_k, b_v, b_gate, b_fc)
    )

    if use_bias not in _CACHE:
        _CACHE[use_bias] = build_nc(use_bias)
    nc = _CACHE[use_bias]

    in_maps = _prep_inputs(
        enc_input, non_pad_mask, slf_attn_mask,
        w_q, b_q, w_k, b_k, w_v, b_v, w_gate, b_gate, w_fc, b_fc, use_bias,
    )
    res = bass_utils.run_bass_kernel_spmd(nc, in_maps, core_ids=list(range(NCORES)))
    out = np.stack([res.results[b]["y"] for b in range(B)], axis=0)
    return out.astype(np.float32)
